# revision 3
# baseline (speedup 1.0000x reference)
"""Per-row cosine similarity kernel for Trainium2 (Bass/Tile), 8-core SPMD.

Problem: a, b: [64, 2048, 512] fp32 -> out [64, 2048] fp32
  out[i,t] = sum(an*bn, -1),  an = a/||a||_clamped, bn = b/||b||_clamped
           = dot(a,b) / (sqrt(max(|a|^2,eps)) * sqrt(max(|b|^2,eps)))

Sharding: 131072 rows split into 8 contiguous blocks of 16384 rows, one per
NeuronCore (data parallel, no communication).

Per-core layout: rows viewed as [128 partitions, 128 tiles, 512] with
row = p*128 + t, so the [128,128] stats tiles map to a contiguous output.
Engines:
  - DVE  : fused tensor_tensor_reduce for dot(a,b) and ~2/3 of the |a|^2 sums
  - ACT  : activation(Square, accum_out=...) for |b|^2 and ~1/3 of |a|^2
  - DMA  : 2 MB chunk loads (16 KB contiguous per partition)
This balances DVE/ACT at ~140 us/core, below the ~190 us HBM roofline for
the 67 MB/core of reads (memory-bound regime).
"""

import os
import sys

import numpy as np

sys.path.insert(0, "/opt/trn_rl_repo")

import concourse.bacc as bacc
import concourse.bass as bass
import concourse.mybir as mybir
import concourse.tile as tile
from concourse.bass_utils import run_bass_kernel_spmd

N_CORES = 8
B, T, D = 64, 2048, 512
ROWS_TOTAL = B * T            # 131072
ROWS_PER_CORE = ROWS_TOTAL // N_CORES  # 16384
P = 128                        # SBUF partitions
T_PER_CORE = ROWS_PER_CORE // P  # 128 stats columns per core
CHUNK_T = 8                    # sub-tiles per DMA chunk (16 KB/partition)
N_CHUNKS = T_PER_CORE // CHUNK_T
EPS = 1e-12

F32 = mybir.dt.float32
MULT = mybir.AluOpType.mult
ADD = mybir.AluOpType.add


def _build():
    nc = bacc.Bacc(
        "TRN2",
        target_bir_lowering=False,
        debug=False,
        enable_asserts=False,
        num_devices=N_CORES,
    )
    a = nc.dram_tensor("a", [ROWS_PER_CORE, D], F32, kind="ExternalInput").ap()
    b = nc.dram_tensor("b", [ROWS_PER_CORE, D], F32, kind="ExternalInput").ap()
    o = nc.dram_tensor("o", [ROWS_PER_CORE], F32, kind="ExternalOutput").ap()

    a_v = a.rearrange("(p t) d -> p t d", p=P)
    b_v = b.rearrange("(p t) d -> p t d", p=P)
    o_v = o.rearrange("(p t) -> p t", p=P)

    with tile.TileContext(nc) as tc:
        with (
            tc.tile_pool(name="io", bufs=3) as io_pool,
            tc.tile_pool(name="scr", bufs=4) as scr_pool,
            tc.tile_pool(name="stats", bufs=1) as stats_pool,
            tc.tile_pool(name="fin", bufs=1) as fin_pool,
        ):
            dot_s = stats_pool.tile([P, T_PER_CORE], F32, tag="dot")
            na_s = stats_pool.tile([P, T_PER_CORE], F32, tag="na")
            nb_s = stats_pool.tile([P, T_PER_CORE], F32, tag="nb")

            for c in range(N_CHUNKS):
                a_t = io_pool.tile([P, CHUNK_T * D], F32, tag="a")
                b_t = io_pool.tile([P, CHUNK_T * D], F32, tag="b")
                nc.sync.dma_start(a_t[:], a_v[:, c * CHUNK_T:(c + 1) * CHUNK_T, :])
                nc.sync.dma_start(b_t[:], b_v[:, c * CHUNK_T:(c + 1) * CHUNK_T, :])
                for k in range(CHUNK_T):
                    g = c * CHUNK_T + k
                    sl = slice(k * D, (k + 1) * D)
                    scr_dot = scr_pool.tile([P, D], F32, tag="scr_dot")
                    nc.vector.affine_mul_reduce(
                        out=scr_dot[:],
                        accum_out=dot_s[:, g:g + 1],
                        in0=a_t[:, sl],
                        in1=b_t[:, sl],
                        scale=1.0,
                        bias=0.0,
                    )
                    # |a|^2: 2 of 3 tiles on DVE (fused TTR), 1 of 3 on ACT,
                    # balancing engine time against the ACT-only |b|^2 pass.
                    if g % 3 == 2:
                        scr_a = scr_pool.tile([P, D], F32, tag="scr_a_act")
                        nc.scalar.activation(
                            scr_a[:],
                            a_t[:, sl],
                            mybir.ActivationFunctionType.Square,
                            accum_out=na_s[:, g:g + 1],
                        )
                    else:
                        scr_a = scr_pool.tile([P, D], F32, tag="scr_a_dve")
                        nc.vector.affine_mul_reduce(
                            out=scr_a[:],
                            accum_out=na_s[:, g:g + 1],
                            in0=a_t[:, sl],
                            in1=a_t[:, sl],
                            scale=1.0,
                            bias=0.0,
                        )
                    scr_b = scr_pool.tile([P, D], F32, tag="scr_b")
                    nc.scalar.activation(
                        scr_b[:],
                        b_t[:, sl],
                        mybir.ActivationFunctionType.Square,
                        accum_out=nb_s[:, g:g + 1],
                    )

            # Final combine: out = dot / sqrt(max(na,eps) * max(nb,eps))
            na_c = fin_pool.tile([P, T_PER_CORE], F32, tag="na_c")
            nb_c = fin_pool.tile([P, T_PER_CORE], F32, tag="nb_c")
            nc.vector.tensor_scalar_max(na_c[:], na_s[:], EPS)
            nc.vector.tensor_scalar_max(nb_c[:], nb_s[:], EPS)
            prod = fin_pool.tile([P, T_PER_CORE], F32, tag="prod")
            nc.vector.tensor_mul(prod[:], na_c[:], nb_c[:])
            rt = fin_pool.tile([P, T_PER_CORE], F32, tag="rt")
            nc.scalar.sqrt(rt[:], prod[:])
            inv = fin_pool.tile([P, T_PER_CORE], F32, tag="inv")
            nc.vector.reciprocal(inv[:], rt[:])
            res = fin_pool.tile([P, T_PER_CORE], F32, tag="res")
            nc.vector.tensor_mul(res[:], dot_s[:], inv[:])
            nc.sync.dma_start(o_v, res[:])

    nc.compile()
    return nc


_NC = None


def _get_nc():
    global _NC
    if _NC is None:
        _NC = _build()
    return _NC


def kernel(a: np.ndarray, b: np.ndarray, _trace: bool = False):
    nc = _get_nc()
    af = np.ascontiguousarray(np.asarray(a, dtype=np.float32).reshape(ROWS_TOTAL, D))
    bf = np.ascontiguousarray(np.asarray(b, dtype=np.float32).reshape(ROWS_TOTAL, D))
    in_maps = [
        {
            "a": af[i * ROWS_PER_CORE:(i + 1) * ROWS_PER_CORE],
            "b": bf[i * ROWS_PER_CORE:(i + 1) * ROWS_PER_CORE],
        }
        for i in range(N_CORES)
    ]
    results = run_bass_kernel_spmd(
        nc, in_maps, core_ids=list(range(N_CORES)), trace=_trace
    )
    out = np.concatenate([results.results[i]["o"] for i in range(N_CORES)])
    if _trace:
        kernel.last_results = results
    return out.reshape(B, T).astype(np.float32)


# revision 7
# speedup vs baseline: 1.0397x; 1.0397x over previous
"""Per-row cosine similarity kernel for Trainium2 (Bass/Tile), 8-core SPMD.

Problem: a, b: [64, 2048, 512] fp32 -> out [64, 2048] fp32
  out[i,t] = sum(an*bn, -1),  an = a/||a||_clamped, bn = b/||b||_clamped
           = dot(a,b) / (sqrt(max(|a|^2,eps)) * sqrt(max(|b|^2,eps)))

Sharding: 131072 rows split into 8 contiguous blocks of 16384 rows, one per
NeuronCore (data parallel, no communication).

Per-core layout: rows viewed as [128 partitions, 128 tiles, 512] with
row = p*128 + t, so the [128,128] stats tiles map to a contiguous output.
Engines:
  - DVE  : fused tensor_tensor_reduce for dot(a,b) and ~2/3 of the |a|^2 sums
  - ACT  : activation(Square, accum_out=...) for |b|^2 and ~1/3 of |a|^2
  - DMA  : 2 MB chunk loads (16 KB contiguous per partition)
This balances DVE/ACT at ~140 us/core, below the ~190 us HBM roofline for
the 67 MB/core of reads (memory-bound regime).
"""

import os
import sys

import numpy as np

sys.path.insert(0, "/opt/trn_rl_repo")

import concourse.bacc as bacc
import concourse.bass as bass
import concourse.mybir as mybir
import concourse.tile as tile
from concourse.bass_utils import run_bass_kernel_spmd

N_CORES = 8
B, T, D = 64, 2048, 512
ROWS_TOTAL = B * T            # 131072
ROWS_PER_CORE = ROWS_TOTAL // N_CORES  # 16384
P = 128                        # SBUF partitions
T_PER_CORE = ROWS_PER_CORE // P  # 128 stats columns per core
CHUNK_T = 4                    # sub-tiles per DMA chunk (8 KB/partition)
N_CHUNKS = T_PER_CORE // CHUNK_T
IO_BUFS = 5                    # prefetch depth (chunks in flight)
GROUP = 32                     # stats columns per pipelined combine
EPS = 1e-12

F32 = mybir.dt.float32
MULT = mybir.AluOpType.mult
ADD = mybir.AluOpType.add


def _build():
    nc = bacc.Bacc(
        "TRN2",
        target_bir_lowering=False,
        debug=False,
        enable_asserts=False,
        num_devices=N_CORES,
    )
    a = nc.dram_tensor("a", [ROWS_PER_CORE, D], F32, kind="ExternalInput").ap()
    b = nc.dram_tensor("b", [ROWS_PER_CORE, D], F32, kind="ExternalInput").ap()
    o = nc.dram_tensor("o", [ROWS_PER_CORE], F32, kind="ExternalOutput").ap()

    a_v = a.rearrange("(p t) d -> p t d", p=P)
    b_v = b.rearrange("(p t) d -> p t d", p=P)
    o_v = o.rearrange("(p t) -> p t", p=P)

    with tile.TileContext(nc) as tc:
        with (
            tc.tile_pool(name="io", bufs=IO_BUFS) as io_pool,
            tc.tile_pool(name="scr", bufs=4) as scr_pool,
            tc.tile_pool(name="stats", bufs=1) as stats_pool,
            tc.tile_pool(name="fin", bufs=1) as fin_pool,
        ):
            dot_s = stats_pool.tile([P, T_PER_CORE], F32, tag="dot")
            na_s = stats_pool.tile([P, T_PER_CORE], F32, tag="na")
            nb_s = stats_pool.tile([P, T_PER_CORE], F32, tag="nb")

            for c in range(N_CHUNKS):
                a_t = io_pool.tile([P, CHUNK_T * D], F32, tag="a")
                b_t = io_pool.tile([P, CHUNK_T * D], F32, tag="b")
                nc.sync.dma_start(a_t[:], a_v[:, c * CHUNK_T:(c + 1) * CHUNK_T, :])
                nc.sync.dma_start(b_t[:], b_v[:, c * CHUNK_T:(c + 1) * CHUNK_T, :])
                for k in range(CHUNK_T):
                    g = c * CHUNK_T + k
                    sl = slice(k * D, (k + 1) * D)
                    scr_dot = scr_pool.tile([P, D], F32, tag="scr_dot")
                    nc.vector.affine_mul_reduce(
                        out=scr_dot[:],
                        accum_out=dot_s[:, g:g + 1],
                        in0=a_t[:, sl],
                        in1=b_t[:, sl],
                        scale=1.0,
                        bias=0.0,
                    )
                    # |a|^2: 2 of 3 tiles on DVE (fused TTR), 1 of 3 on ACT,
                    # balancing engine time against the ACT-only |b|^2 pass.
                    if g % 3 == 2:
                        scr_a = scr_pool.tile([P, D], F32, tag="scr_a_act")
                        nc.scalar.activation(
                            scr_a[:],
                            a_t[:, sl],
                            mybir.ActivationFunctionType.Square,
                            accum_out=na_s[:, g:g + 1],
                        )
                    else:
                        scr_a = scr_pool.tile([P, D], F32, tag="scr_a_dve")
                        nc.vector.affine_mul_reduce(
                            out=scr_a[:],
                            accum_out=na_s[:, g:g + 1],
                            in0=a_t[:, sl],
                            in1=a_t[:, sl],
                            scale=1.0,
                            bias=0.0,
                        )
                    scr_b = scr_pool.tile([P, D], F32, tag="scr_b")
                    nc.scalar.activation(
                        scr_b[:],
                        b_t[:, sl],
                        mybir.ActivationFunctionType.Square,
                        accum_out=nb_s[:, g:g + 1],
                    )

                # Pipelined final combine: once a group of stats columns is
                # complete, compute out = dot / sqrt(max(na,eps)*max(nb,eps))
                # for those columns and store. Hides the combine + output DMA
                # under the remaining chunk loads (only the last group is on
                # the critical tail). Square and Sqrt share an ACT table set
                # (sqrt_and_others), so no table reloads.
                if (c + 1) * CHUNK_T % GROUP == 0:
                    hi = (c + 1) * CHUNK_T
                    lo = hi - GROUP
                    gs = slice(lo, hi)
                    na_c = fin_pool.tile([P, GROUP], F32, tag="na_c")
                    nb_c = fin_pool.tile([P, GROUP], F32, tag="nb_c")
                    nc.vector.tensor_scalar_max(na_c[:], na_s[:, gs], EPS)
                    nc.vector.tensor_scalar_max(nb_c[:], nb_s[:, gs], EPS)
                    prod = fin_pool.tile([P, GROUP], F32, tag="prod")
                    nc.vector.tensor_mul(prod[:], na_c[:], nb_c[:])
                    rt = fin_pool.tile([P, GROUP], F32, tag="rt")
                    nc.scalar.sqrt(rt[:], prod[:])
                    inv = fin_pool.tile([P, GROUP], F32, tag="inv")
                    nc.vector.reciprocal(inv[:], rt[:])
                    res = fin_pool.tile([P, GROUP], F32, tag="res")
                    nc.vector.tensor_mul(res[:], dot_s[:, gs], inv[:])
                    nc.sync.dma_start(o_v[:, gs], res[:])

    nc.compile()
    return nc


_NC = None


def _get_nc():
    global _NC
    if _NC is None:
        _NC = _build()
    return _NC


def kernel(a: np.ndarray, b: np.ndarray, _trace: bool = False):
    nc = _get_nc()
    af = np.ascontiguousarray(np.asarray(a, dtype=np.float32).reshape(ROWS_TOTAL, D))
    bf = np.ascontiguousarray(np.asarray(b, dtype=np.float32).reshape(ROWS_TOTAL, D))
    in_maps = [
        {
            "a": af[i * ROWS_PER_CORE:(i + 1) * ROWS_PER_CORE],
            "b": bf[i * ROWS_PER_CORE:(i + 1) * ROWS_PER_CORE],
        }
        for i in range(N_CORES)
    ]
    results = run_bass_kernel_spmd(
        nc, in_maps, core_ids=list(range(N_CORES)), trace=_trace
    )
    out = np.concatenate([results.results[i]["o"] for i in range(N_CORES)])
    if _trace:
        kernel.last_results = results
    return out.reshape(B, T).astype(np.float32)
